# revision 11
# baseline (speedup 1.0000x reference)
"""Trainium2 Bass kernel for BaselineDNN, W1-folded variant.

  logits[b] = relu((sum_l emb[x[b,l]]) / len[b] @ W1 + b1) @ W2 + b2

Key algebraic fold: pooling and the first MLP layer are both linear, so
  (sum_l emb[x[b,l]]) @ W1 = sum_l (emb @ W1)[x[b,l]].
The host precomputes emb2 = emb_table @ W1 [50000, 32] and the device
gathers 256B rows ([128] fp16, first 32 valid) instead of 768B rows of the
raw table — 3x less HBM gather traffic, same descriptor count.

Sharding: data-parallel over batch. Each of the 8 cores handles B/8 = 256
batch rows; the folded table and tiny tail weights are replicated. One SPMD
program runs on all 8 cores.

Gather: the dma_gather primitive takes int16 (signed) row indices, so the
50002-row table needs two base views:
  lo: rows [0, 32768)      idx = x + 1       (x <= 32766), filler idx 0
  hi: rows [17234, 50002)  idx = x - 17233   (x >= 32767), filler idx 32767
Row 0 and row 50001 are zero rows (fillers). Per batch row the 200 tokens
are partitioned host-side (order-invariant under the sum) into a lo-list
and a hi-list. Rows are globally sorted by lo-count into 16 narrow-spread
tiles of 128 (tile g pairs with tile 15-g on a core to balance work); the
host inverse-permutes the output.

HW datapath facts (microbenched): gather throughput is per-SWDGE-queue
bound at ~8.1 ns/descriptor and scales linearly up to the 4-queue ucode
max; address randomness and element size are second-order. So the gather
plan is: per core, split each tile's lo/hi block into 2 equal column
chunks -> 8 gathers, greedily assigned to the 4 queues so every queue
carries an equal descriptor load. Ring scratch is 64KB/partition so a
~80-column gather (5 descs/ring-slot..) fits its ring without stalling
descriptor generation.

Pooling: per valid token-slot, a TensorE matmul with a 128x128 fp16
identity accumulates the [128, 32] slot slice into fp32 PSUM. ScalarE
scales by 1/len, TensorE transposes to [32, 128], ScalarE applies b1/relu,
TensorE runs W2, ScalarE adds b2. Logits are written transposed [3, 256]
per core; the host reassembles and un-permutes [2048, 3].
"""

import numpy as np
from contextlib import ExitStack

import concourse.bass as bass
import concourse.bacc as bacc
import concourse.mybir as mybir
import concourse.tile as tile
from concourse.bass_utils import run_bass_kernel_spmd
from concourse.masks import make_identity

# Problem shapes (hardcoded per spec)
B, L, V, D, H, C = 2048, 200, 50000, 300, 32, 3
N_CORES = 8
BS = B // N_CORES   # 256 batch rows per core
P = 128             # partitions
N_TILES = BS // P   # batch tiles per core
E = 128             # folded fp16 row (256B, the dma_gather minimum)

N_SPLIT = 2            # chunks per (tile, lo/hi) block
SINGLE_PACKET = False  # True crashes the runtime on multi-packet gathers
DMA_SCRATCH = 65536    # SWDGE descriptor-ring carveout bytes/partition
N_QUEUES = 4           # SWDGE queues used (ucode max)
GBUFS = 4              # in-flight gather buffers

LO_BASE = 0         # lo view: table rows [0, 32768)
HI_BASE = 17234     # hi view: table rows [17234, 50002)
LO_FILL = 0         # zero row (table row 0)
HI_FILL = 32767     # zero row (table row 50001)
X_SPLIT = 32767     # x < split -> lo (idx x+1); x >= split -> hi (idx x-17233)
FILL_SENTINEL = -9999

F32 = mybir.dt.float32
F16 = mybir.dt.float16
I16 = mybir.dt.int16

_CACHE = {}


def _split_cols(total, n):
    """Split `total` columns into n near-equal positive chunks."""
    out = []
    for i in range(n):
        c = (total + (n - i) - 1) // (n - i)
        out.append(c)
        total -= c
    return [c for c in out if c > 0]


def _plan_jobs(cols):
    """cols[t] = (lo_cols, hi_cols) per tile. Returns a list of gather jobs
    (t, kind, col0, ncols, queue) in issue order, queues balanced by
    descriptor load."""
    jobs = []
    for t, (lo_c, hi_c) in enumerate(cols):
        for kind, tot in (("lo", lo_c), ("hi", hi_c)):
            c0 = 0
            for c in _split_cols(tot, N_SPLIT):
                jobs.append([t, kind, c0, c])
                c0 += c
    # greedy balance: biggest first onto the lightest queue
    order = sorted(range(len(jobs)), key=lambda j: -jobs[j][3])
    qload = [0] * N_QUEUES
    qassign = {}
    for j in order:
        q = min(range(N_QUEUES), key=lambda k: qload[k])
        qassign[j] = q
        qload[q] += jobs[j][3]
    # issue order: round-robin across queues so all start busy
    byq = [[j for j in range(len(jobs)) if qassign[j] == k] for k in range(N_QUEUES)]
    issue = []
    i = 0
    while any(byq):
        for k in range(N_QUEUES):
            if byq[k]:
                issue.append(byq[k].pop(0))
        i += 1
    return [(jobs[j][0], jobs[j][1], jobs[j][2], jobs[j][3], qassign[j])
            for j in issue], qload


def _build_nc(cols, reps=1):
    """cols[t] = (lo_cols, hi_cols): equalized valid 128-index columns per
    tile for the lo and hi blocks. Identical across cores by construction."""
    jobs, qload = _plan_jobs(cols)
    idx_cols = [8 * (lo + hi) for lo, hi in cols]

    nc = bacc.Bacc("TRN2", debug=False, num_devices=N_CORES,
                   num_swdge_queues=N_QUEUES,
                   dynamic_dma_scratch_size=DMA_SCRATCH)

    idx_ins = [
        nc.declare_dram_parameter(f"idx{t}", [P, idx_cols[t]], I16,
                                  isOutput=False)
        for t in range(N_TILES)
    ]
    len_in = nc.declare_dram_parameter("lens", [BS, 1], F32, isOutput=False)
    emb_in = nc.declare_dram_parameter("emb", [V + 2, E], F16, isOutput=False)
    b1_in = nc.declare_dram_parameter("b1", [H, 1], F32, isOutput=False)
    w2_in = nc.declare_dram_parameter("w2", [H, C], F32, isOutput=False)
    b2_in = nc.declare_dram_parameter("b2", [C, 1], F32, isOutput=False)
    out_dram = nc.declare_dram_parameter("out", [C, BS], F32, isOutput=True)

    emb_lo = emb_in[LO_BASE:LO_BASE + 32768, :]
    emb_hi = emb_in[HI_BASE:HI_BASE + 32768, :]

    with tile.TileContext(nc) as tc, ExitStack() as ctx:
        const_pool = ctx.enter_context(tc.tile_pool(name="const", bufs=1))
        xpool = ctx.enter_context(tc.tile_pool(name="xp", bufs=2))
        gpool = ctx.enter_context(tc.tile_pool(name="gp", bufs=GBUFS))
        spool = ctx.enter_context(tc.tile_pool(name="sp", bufs=2))
        psum_pool = ctx.enter_context(tc.tile_pool(name="ps", bufs=1, space="PSUM"))
        psum_acc = ctx.enter_context(tc.tile_pool(name="psacc", bufs=1, space="PSUM"))

        ident = const_pool.tile([P, P], F32)
        make_identity(nc, ident[:])
        ident16 = const_pool.tile([P, P], F16)
        make_identity(nc, ident16[:])
        b1_sb = const_pool.tile([H, 1], F32)
        nc.sync.dma_start(b1_sb[:], b1_in[:])
        w2_sb = const_pool.tile([H, C], F32)
        nc.sync.dma_start(w2_sb[:], w2_in[:])
        b2_sb = const_pool.tile([C, 1], F32)
        nc.sync.dma_start(b2_sb[:], b2_in[:])

        loop_ctx = tc.For_i(0, reps, 1) if reps > 1 else None
        if loop_ctx is not None:
            ctx.enter_context(loop_ctx)

        idx_t, lens_t, inv_t, acc, n_done = {}, {}, {}, {}, {}
        for t in range(N_TILES):
            r0 = t * P
            xt = xpool.tile([P, idx_cols[t]], I16, tag=f"xt{t}", name=f"xt{t}")
            nc.sync.dma_start(xt[:], idx_ins[t][:, :])
            idx_t[t] = xt
            lt = xpool.tile([P, 1], F32, tag=f"lt{t}", name=f"lt{t}")
            nc.sync.dma_start(lt[:], len_in[r0:r0 + P, :])
            it = xpool.tile([P, 1], F32, tag=f"it{t}", name=f"it{t}")
            nc.vector.reciprocal(it[:], lt[:])
            inv_t[t] = it
            acc[t] = psum_acc.tile([P, H], F32, tag=f"acc{t}", name=f"acc{t}")
            n_done[t] = 0

        maxc = max(j[3] for j in jobs)
        for (t, kind, c0, ncols, q) in jobs:
            lo_c, hi_c = cols[t]
            n_valid_tot = lo_c + hi_c
            src = emb_lo if kind == "lo" else emb_hi
            # column offset of this job inside the tile's idx tile
            base = 0 if kind == "lo" else lo_c
            col0 = 8 * (base + c0)
            g = gpool.tile([P, maxc * E], F16, tag="g", name="g")
            gv = g[:, :ncols * E].rearrange("p (c e) -> p c e", c=ncols, e=E)
            nc.gpsimd.dma_gather(
                out_ap=gv,
                in_ap=src,
                idxs_ap=idx_t[t][:, col0:col0 + 8 * ncols],
                num_idxs=P * ncols,
                num_idxs_reg=P * ncols,
                elem_size=E,
                single_packet=SINGLE_PACKET,
                queue_num=q,
            )
            for k in range(ncols):
                nc.tensor.matmul(
                    out=acc[t][:],
                    lhsT=ident16[:],
                    rhs=gv[:, k, 0:H],
                    start=(n_done[t] == 0),
                    stop=(n_done[t] == n_valid_tot - 1),
                )
                n_done[t] += 1

        for t in range(N_TILES):
            r0 = t * P
            # rep = acc / len  (ScalarE: PSUM -> SBUF with per-partition scale)
            rep = spool.tile([P, H], F32, tag=f"rep{t}", name=f"rep{t}")
            nc.scalar.mul(rep[:], acc[t][:], inv_t[t][:, :1])

            # transpose to [H, P], then h = relu(rep + b1)
            tp = psum_pool.tile([P, P], F32, tag=f"tp{t}", name=f"tp{t}")
            nc.tensor.transpose(tp[:H, :], rep[:, 0:H], ident[:])
            h_sb = spool.tile([H, P], F32, tag=f"hsb{t}", name=f"hsb{t}")
            nc.scalar.activation(
                h_sb[:], tp[:H, :], mybir.ActivationFunctionType.Relu,
                bias=b1_sb[:, :1], scale=1.0,
            )

            # logits = h @ W2 + b2, as [C, P]
            o_psum = psum_pool.tile([C, P], F32, tag=f"o{t}", name=f"o{t}")
            nc.tensor.matmul(out=o_psum[:], lhsT=w2_sb[:], rhs=h_sb[:],
                             start=True, stop=True)
            logits_sb = spool.tile([C, P], F32, tag=f"lg{t}", name=f"lg{t}")
            nc.scalar.activation(
                logits_sb[:], o_psum[:], mybir.ActivationFunctionType.Identity,
                bias=b2_sb[:, :1], scale=1.0,
            )
            nc.sync.dma_start(out_dram[:, r0:r0 + P], logits_sb[:])

    nc.finalize()
    return nc


def _block_last(vals):
    """Last real flat position + 1 for `vals` ([P, K] with FILL_SENTINEL
    marking fillers), in j = col*128 + p order."""
    real = vals != FILL_SENTINEL
    if not real.any():
        return 0
    cc, pp = np.nonzero(real.T)
    return int((cc * P + pp).max() + 1)


def _wrap_block(blk, lead_fill, n_valid):
    """[P, C] block -> [P, 8*C] wrapped+replicated int16 idx tile.

    dma_gather maps flat index j -> partition j%128, column-group j//128,
    reading the flat list wrapped over 16 partitions (element j at partition
    j%16, column j//16), replicated across the eight 16-partition groups.

    Positions < n_valid that are fillers point at a zero row; positions
    >= n_valid are -1 (trimmed by the Q7: no descriptors, no traffic).
    """
    p, c = blk.shape
    flat = blk.T.reshape(-1).astype(np.int32).copy()
    flat[flat == FILL_SENTINEL] = lead_fill
    flat[n_valid:] = -1
    flat = flat.astype(np.int16)
    w = flat.reshape(8 * c, 16).T           # [16, 8*c]: element j at (j%16, j//16)
    return np.tile(w, (8, 1))               # replicate to 128 partitions


def _prep_idx(x32):
    """Split tokens lo/hi per row, globally sort rows by lo-count into
    narrow-spread tiles, equalize per-(tile-slot, block) valid columns
    across cores, and build wrapped idx tiles.

    Returns (idx arrays per core: list over tiles of [P, 8*(lo+hi)],
    cols, row_order) where row_order[c*BS + i] is the original batch row
    handled by core c, slot i.
    """
    is_hi = x32 >= X_SPLIT
    n_lo = (~is_hi).sum(axis=1)                       # [B]
    k_lo = int(n_lo.max())
    k_hi = int((L - n_lo).max())
    order = np.argsort(is_hi, axis=1, kind="stable")  # lo positions first
    xo = np.take_along_axis(x32, order, axis=1)       # [B, L] lo tokens then hi

    colsr = np.arange(L)[None, :]
    lo_vals = np.where(colsr < n_lo[:, None], xo + 1, FILL_SENTINEL)
    hi_src = np.take_along_axis(
        xo, np.minimum(colsr + n_lo[:, None], L - 1), axis=1)
    hi_vals = np.where(colsr < (L - n_lo)[:, None], hi_src - 17233, FILL_SENTINEL)
    lo_all = lo_vals[:, :k_lo]
    hi_all = hi_vals[:, :k_hi]

    # Global sort by n_lo -> 16 tiles of 128 rows with narrow n_lo spread;
    # within a tile sort descending so block tails are maximally trimmable.
    # Pair tile g with tile 15-g on one core to balance per-core work.
    gorder = np.argsort(n_lo, kind="stable")
    n_gtiles = B // P
    gtiles = [gorder[i * P:(i + 1) * P] for i in range(n_gtiles)]
    gtiles = [t[np.argsort(-n_lo[t], kind="stable")] for t in gtiles]

    tile_rows = {}
    for c in range(N_CORES):
        for t, g in enumerate([c, n_gtiles - 1 - c]):
            tile_rows[(c, t)] = gtiles[g]

    # Equalized valid column counts per (tile-slot, lo/hi): max over cores,
    # rounded up to a full 128-index column so every gathered column is
    # completely written (pooling matmuls touch every valid column).
    cols = []
    for t in range(N_TILES):
        pair = []
        for vals_all in (lo_all, hi_all):
            m = 0
            for c in range(N_CORES):
                rows = tile_rows[(c, t)]
                m = max(m, _block_last(vals_all[rows]))
            pair.append((m + P - 1) // P)
        cols.append(tuple(pair))
    cols = tuple(cols)

    idx_per_core = []
    row_order = np.empty(B, dtype=np.int64)
    for c in range(N_CORES):
        tiles = []
        for t in range(N_TILES):
            rows = tile_rows[(c, t)]
            row_order[c * BS + t * P:c * BS + (t + 1) * P] = rows
            lo_c, hi_c = cols[t]
            blocks = [
                _wrap_block(lo_all[rows][:, :lo_c], LO_FILL, lo_c * P),
                _wrap_block(hi_all[rows][:, :hi_c], HI_FILL, hi_c * P),
            ]
            tiles.append(np.concatenate(blocks, axis=1))
        idx_per_core.append([np.ascontiguousarray(a) for a in tiles])
    return idx_per_core, cols, row_order


def _prep_inputs(x, lengths, emb_table, W1, b1, W2, b2):
    x32 = np.asarray(x).astype(np.int32)
    idx_per_core, cols, row_order = _prep_idx(x32)

    lens = np.ascontiguousarray(
        np.asarray(lengths).astype(np.float32).reshape(B, 1)[row_order])
    # Fold W1 into the table: emb2 = emb @ W1 [V, H], padded fp16 to 256B rows
    emb2 = np.asarray(emb_table, dtype=np.float32) @ np.asarray(W1, dtype=np.float32)
    emb_p = np.zeros((V + 2, E), dtype=np.float16)
    emb_p[1:V + 1, :H] = emb2.astype(np.float16)
    b1c = np.ascontiguousarray(np.asarray(b1, dtype=np.float32).reshape(H, 1))
    w2 = np.ascontiguousarray(np.asarray(W2, dtype=np.float32))
    b2c = np.ascontiguousarray(np.asarray(b2, dtype=np.float32).reshape(C, 1))
    in_maps = [
        {
            **{f"idx{t}": idx_per_core[c][t] for t in range(N_TILES)},
            "lens": lens[c * BS:(c + 1) * BS],
            "emb": emb_p,
            "b1": b1c,
            "w2": w2,
            "b2": b2c,
        }
        for c in range(N_CORES)
    ]
    return in_maps, cols, row_order


def run_on_device(in_maps, cols, **kwargs):
    if _CACHE.get("key") != cols:
        _CACHE["nc"] = _build_nc(cols)
        _CACHE["key"] = cols
    return run_bass_kernel_spmd(_CACHE["nc"], in_maps, list(range(N_CORES)),
                                **kwargs)


def kernel(x, lengths, emb_table, W1, b1, W2, b2):
    in_maps, cols, row_order = _prep_inputs(
        x, lengths, emb_table, W1, b1, W2, b2)
    res = run_on_device(in_maps, cols)
    out = np.concatenate([r["out"] for r in res.results], axis=1)  # [C, B]
    full = np.empty((B, C), dtype=np.float32)
    full[row_order] = out.T  # undo the global row sort
    return full


# revision 12
# speedup vs baseline: 1.2905x; 1.2905x over previous
"""Trainium2 Bass kernel for BaselineDNN, W1-folded variant.

  logits[b] = relu((sum_l emb[x[b,l]]) / len[b] @ W1 + b1) @ W2 + b2

Key algebraic fold: pooling and the first MLP layer are both linear, so
  (sum_l emb[x[b,l]]) @ W1 = sum_l (emb @ W1)[x[b,l]].
The host precomputes emb2 = emb_table @ W1 [50000, 32] and the device
gathers 256B rows ([128] fp16, first 32 valid) instead of 768B rows of the
raw table — 3x less HBM gather traffic, same descriptor count.

Sharding: data-parallel over batch. Each of the 8 cores handles B/8 = 256
batch rows; the folded table and tiny tail weights are replicated. One SPMD
program runs on all 8 cores.

Gather: the dma_gather primitive takes int16 (signed) row indices, so the
50002-row table needs two base views:
  lo: rows [0, 32768)      idx = x + 1       (x <= 32766), filler idx 0
  hi: rows [17234, 50002)  idx = x - 17233   (x >= 32767), filler idx 32767
Row 0 and row 50001 are zero rows (fillers). Per batch row the 200 tokens
are partitioned host-side (order-invariant under the sum) into a lo-list
and a hi-list. Rows are globally sorted by lo-count into 16 narrow-spread
tiles of 128 (tile g pairs with tile 15-g on a core to balance work); the
host inverse-permutes the output.

HW datapath facts (microbenched): gather throughput is per-SWDGE-queue
bound at ~8.1 ns/descriptor and scales linearly up to the 4-queue ucode
max; address randomness and element size are second-order. So the gather
plan is: per core, split each tile's lo/hi block into 2 equal column
chunks -> 8 gathers, greedily assigned to the 4 queues so every queue
carries an equal descriptor load. Ring scratch is 64KB/partition so a
~80-column gather (5 descs/ring-slot..) fits its ring without stalling
descriptor generation.

Pooling: per valid token-slot, a TensorE matmul with a 128x128 fp16
identity accumulates the [128, 32] slot slice into fp32 PSUM. ScalarE
scales by 1/len, TensorE transposes to [32, 128], ScalarE applies b1/relu,
TensorE runs W2, ScalarE adds b2. Logits are written transposed [3, 256]
per core; the host reassembles and un-permutes [2048, 3].
"""

import numpy as np
from contextlib import ExitStack

import concourse.bass as bass
import concourse.bacc as bacc
import concourse.mybir as mybir
import concourse.tile as tile
from concourse.bass_utils import run_bass_kernel_spmd
from concourse.masks import make_identity

# Problem shapes (hardcoded per spec)
B, L, V, D, H, C = 2048, 200, 50000, 300, 32, 3
N_CORES = 8
BS = B // N_CORES   # 256 batch rows per core
P = 128             # partitions
N_TILES = BS // P   # batch tiles per core
E = 128             # folded fp16 row (256B, the dma_gather minimum)

N_SPLIT = 4            # chunks per (tile, lo/hi) block
SINGLE_PACKET = False  # True crashes the runtime on multi-packet gathers
DMA_SCRATCH = 32768    # SWDGE descriptor-ring carveout bytes/partition
N_QUEUES = 4           # SWDGE queues used (ucode max)
GBUFS = 8              # in-flight gather buffers

LO_BASE = 0         # lo view: table rows [0, 32768)
HI_BASE = 17234     # hi view: table rows [17234, 50002)
LO_FILL = 0         # zero row (table row 0)
HI_FILL = 32767     # zero row (table row 50001)
X_SPLIT = 32767     # x < split -> lo (idx x+1); x >= split -> hi (idx x-17233)
FILL_SENTINEL = -9999

F32 = mybir.dt.float32
F16 = mybir.dt.float16
I16 = mybir.dt.int16

_CACHE = {}


def _split_cols(total, n):
    """Split `total` columns into n near-equal positive chunks."""
    out = []
    for i in range(n):
        c = (total + (n - i) - 1) // (n - i)
        out.append(c)
        total -= c
    return [c for c in out if c > 0]


def _plan_jobs(cols):
    """cols[t] = (lo_cols, hi_cols) per tile. Returns a list of gather jobs
    (t, kind, col0, ncols, queue) in issue order, queues balanced by
    descriptor load."""
    jobs = []
    for t, (lo_c, hi_c) in enumerate(cols):
        for kind, tot in (("lo", lo_c), ("hi", hi_c)):
            c0 = 0
            for c in _split_cols(tot, N_SPLIT):
                jobs.append([t, kind, c0, c])
                c0 += c
    # greedy balance: biggest first onto the lightest queue
    order = sorted(range(len(jobs)), key=lambda j: -jobs[j][3])
    qload = [0] * N_QUEUES
    qassign = {}
    for j in order:
        q = min(range(N_QUEUES), key=lambda k: qload[k])
        qassign[j] = q
        qload[q] += jobs[j][3]
    # issue order: round-robin across queues so all start busy
    byq = [[j for j in range(len(jobs)) if qassign[j] == k] for k in range(N_QUEUES)]
    issue = []
    i = 0
    while any(byq):
        for k in range(N_QUEUES):
            if byq[k]:
                issue.append(byq[k].pop(0))
        i += 1
    return [(jobs[j][0], jobs[j][1], jobs[j][2], jobs[j][3], qassign[j])
            for j in issue], qload


def _build_nc(cols, reps=1):
    """cols[t] = (lo_cols, hi_cols): equalized valid 128-index columns per
    tile for the lo and hi blocks. Identical across cores by construction."""
    jobs, qload = _plan_jobs(cols)
    idx_cols = [8 * (lo + hi) for lo, hi in cols]

    nc = bacc.Bacc("TRN2", debug=False, num_devices=N_CORES,
                   num_swdge_queues=N_QUEUES,
                   dynamic_dma_scratch_size=DMA_SCRATCH)

    idx_ins = [
        nc.declare_dram_parameter(f"idx{t}", [P, idx_cols[t]], I16,
                                  isOutput=False)
        for t in range(N_TILES)
    ]
    len_in = nc.declare_dram_parameter("lens", [BS, 1], F32, isOutput=False)
    emb_in = nc.declare_dram_parameter("emb", [V + 2, E], F16, isOutput=False)
    b1_in = nc.declare_dram_parameter("b1", [H, 1], F32, isOutput=False)
    w2_in = nc.declare_dram_parameter("w2", [H, C], F32, isOutput=False)
    b2_in = nc.declare_dram_parameter("b2", [C, 1], F32, isOutput=False)
    out_dram = nc.declare_dram_parameter("out", [C, BS], F32, isOutput=True)

    emb_lo = emb_in[LO_BASE:LO_BASE + 32768, :]
    emb_hi = emb_in[HI_BASE:HI_BASE + 32768, :]

    with tile.TileContext(nc) as tc, ExitStack() as ctx:
        const_pool = ctx.enter_context(tc.tile_pool(name="const", bufs=1))
        xpool = ctx.enter_context(tc.tile_pool(name="xp", bufs=2))
        gpool = ctx.enter_context(tc.tile_pool(name="gp", bufs=GBUFS))
        spool = ctx.enter_context(tc.tile_pool(name="sp", bufs=2))
        psum_pool = ctx.enter_context(tc.tile_pool(name="ps", bufs=1, space="PSUM"))
        psum_acc = ctx.enter_context(tc.tile_pool(name="psacc", bufs=1, space="PSUM"))

        ident = const_pool.tile([P, P], F32)
        make_identity(nc, ident[:])
        ident16 = const_pool.tile([P, P], F16)
        make_identity(nc, ident16[:])
        b1_sb = const_pool.tile([H, 1], F32)
        nc.sync.dma_start(b1_sb[:], b1_in[:])
        w2_sb = const_pool.tile([H, C], F32)
        nc.sync.dma_start(w2_sb[:], w2_in[:])
        b2_sb = const_pool.tile([C, 1], F32)
        nc.sync.dma_start(b2_sb[:], b2_in[:])

        loop_ctx = tc.For_i(0, reps, 1) if reps > 1 else None
        if loop_ctx is not None:
            ctx.enter_context(loop_ctx)

        idx_t, lens_t, inv_t, acc, n_done = {}, {}, {}, {}, {}
        for t in range(N_TILES):
            r0 = t * P
            xt = xpool.tile([P, idx_cols[t]], I16, tag=f"xt{t}", name=f"xt{t}")
            nc.sync.dma_start(xt[:], idx_ins[t][:, :])
            idx_t[t] = xt
            lt = xpool.tile([P, 1], F32, tag=f"lt{t}", name=f"lt{t}")
            nc.sync.dma_start(lt[:], len_in[r0:r0 + P, :])
            it = xpool.tile([P, 1], F32, tag=f"it{t}", name=f"it{t}")
            nc.vector.reciprocal(it[:], lt[:])
            inv_t[t] = it
            acc[t] = psum_acc.tile([P, H], F32, tag=f"acc{t}", name=f"acc{t}")
            n_done[t] = 0

        maxc = max(j[3] for j in jobs)
        for (t, kind, c0, ncols, q) in jobs:
            lo_c, hi_c = cols[t]
            n_valid_tot = lo_c + hi_c
            src = emb_lo if kind == "lo" else emb_hi
            # column offset of this job inside the tile's idx tile
            base = 0 if kind == "lo" else lo_c
            col0 = 8 * (base + c0)
            g = gpool.tile([P, maxc * E], F16, tag="g", name="g")
            gv = g[:, :ncols * E].rearrange("p (c e) -> p c e", c=ncols, e=E)
            nc.gpsimd.dma_gather(
                out_ap=gv,
                in_ap=src,
                idxs_ap=idx_t[t][:, col0:col0 + 8 * ncols],
                num_idxs=P * ncols,
                num_idxs_reg=P * ncols,
                elem_size=E,
                single_packet=SINGLE_PACKET,
                queue_num=q,
            )
            for k in range(ncols):
                nc.tensor.matmul(
                    out=acc[t][:],
                    lhsT=ident16[:],
                    rhs=gv[:, k, 0:H],
                    start=(n_done[t] == 0),
                    stop=(n_done[t] == n_valid_tot - 1),
                )
                n_done[t] += 1

        for t in range(N_TILES):
            r0 = t * P
            # rep = acc / len  (ScalarE: PSUM -> SBUF with per-partition scale)
            rep = spool.tile([P, H], F32, tag=f"rep{t}", name=f"rep{t}")
            nc.scalar.mul(rep[:], acc[t][:], inv_t[t][:, :1])

            # transpose to [H, P], then h = relu(rep + b1)
            tp = psum_pool.tile([P, P], F32, tag=f"tp{t}", name=f"tp{t}")
            nc.tensor.transpose(tp[:H, :], rep[:, 0:H], ident[:])
            h_sb = spool.tile([H, P], F32, tag=f"hsb{t}", name=f"hsb{t}")
            nc.scalar.activation(
                h_sb[:], tp[:H, :], mybir.ActivationFunctionType.Relu,
                bias=b1_sb[:, :1], scale=1.0,
            )

            # logits = h @ W2 + b2, as [C, P]
            o_psum = psum_pool.tile([C, P], F32, tag=f"o{t}", name=f"o{t}")
            nc.tensor.matmul(out=o_psum[:], lhsT=w2_sb[:], rhs=h_sb[:],
                             start=True, stop=True)
            logits_sb = spool.tile([C, P], F32, tag=f"lg{t}", name=f"lg{t}")
            nc.scalar.activation(
                logits_sb[:], o_psum[:], mybir.ActivationFunctionType.Identity,
                bias=b2_sb[:, :1], scale=1.0,
            )
            nc.sync.dma_start(out_dram[:, r0:r0 + P], logits_sb[:])

    nc.finalize()
    return nc


def _block_last(vals):
    """Last real flat position + 1 for `vals` ([P, K] with FILL_SENTINEL
    marking fillers), in j = col*128 + p order."""
    real = vals != FILL_SENTINEL
    if not real.any():
        return 0
    cc, pp = np.nonzero(real.T)
    return int((cc * P + pp).max() + 1)


def _wrap_block(blk, lead_fill, n_valid):
    """[P, C] block -> [P, 8*C] wrapped+replicated int16 idx tile.

    dma_gather maps flat index j -> partition j%128, column-group j//128,
    reading the flat list wrapped over 16 partitions (element j at partition
    j%16, column j//16), replicated across the eight 16-partition groups.

    Positions < n_valid that are fillers point at a zero row; positions
    >= n_valid are -1 (trimmed by the Q7: no descriptors, no traffic).
    """
    p, c = blk.shape
    flat = blk.T.reshape(-1).astype(np.int32).copy()
    flat[flat == FILL_SENTINEL] = lead_fill
    flat[n_valid:] = -1
    flat = flat.astype(np.int16)
    w = flat.reshape(8 * c, 16).T           # [16, 8*c]: element j at (j%16, j//16)
    return np.tile(w, (8, 1))               # replicate to 128 partitions


def _prep_idx(x32):
    """Split tokens lo/hi per row, globally sort rows by lo-count into
    narrow-spread tiles, equalize per-(tile-slot, block) valid columns
    across cores, and build wrapped idx tiles.

    Returns (idx arrays per core: list over tiles of [P, 8*(lo+hi)],
    cols, row_order) where row_order[c*BS + i] is the original batch row
    handled by core c, slot i.
    """
    is_hi = x32 >= X_SPLIT
    n_lo = (~is_hi).sum(axis=1)                       # [B]
    k_lo = int(n_lo.max())
    k_hi = int((L - n_lo).max())
    order = np.argsort(is_hi, axis=1, kind="stable")  # lo positions first
    xo = np.take_along_axis(x32, order, axis=1)       # [B, L] lo tokens then hi

    colsr = np.arange(L)[None, :]
    lo_vals = np.where(colsr < n_lo[:, None], xo + 1, FILL_SENTINEL)
    hi_src = np.take_along_axis(
        xo, np.minimum(colsr + n_lo[:, None], L - 1), axis=1)
    hi_vals = np.where(colsr < (L - n_lo)[:, None], hi_src - 17233, FILL_SENTINEL)
    lo_all = lo_vals[:, :k_lo]
    hi_all = hi_vals[:, :k_hi]

    # Global sort by n_lo -> 16 tiles of 128 rows with narrow n_lo spread;
    # within a tile sort descending so block tails are maximally trimmable.
    # Pair tile g with tile 15-g on one core to balance per-core work.
    gorder = np.argsort(n_lo, kind="stable")
    n_gtiles = B // P
    gtiles = [gorder[i * P:(i + 1) * P] for i in range(n_gtiles)]
    gtiles = [t[np.argsort(-n_lo[t], kind="stable")] for t in gtiles]

    tile_rows = {}
    for c in range(N_CORES):
        for t, g in enumerate([c, n_gtiles - 1 - c]):
            tile_rows[(c, t)] = gtiles[g]

    # Equalized valid column counts per (tile-slot, lo/hi): max over cores,
    # rounded up to a full 128-index column so every gathered column is
    # completely written (pooling matmuls touch every valid column).
    cols = []
    for t in range(N_TILES):
        pair = []
        for vals_all in (lo_all, hi_all):
            m = 0
            for c in range(N_CORES):
                rows = tile_rows[(c, t)]
                m = max(m, _block_last(vals_all[rows]))
            pair.append((m + P - 1) // P)
        cols.append(tuple(pair))
    cols = tuple(cols)

    idx_per_core = []
    row_order = np.empty(B, dtype=np.int64)
    for c in range(N_CORES):
        tiles = []
        for t in range(N_TILES):
            rows = tile_rows[(c, t)]
            row_order[c * BS + t * P:c * BS + (t + 1) * P] = rows
            lo_c, hi_c = cols[t]
            blocks = [
                _wrap_block(lo_all[rows][:, :lo_c], LO_FILL, lo_c * P),
                _wrap_block(hi_all[rows][:, :hi_c], HI_FILL, hi_c * P),
            ]
            tiles.append(np.concatenate(blocks, axis=1))
        idx_per_core.append([np.ascontiguousarray(a) for a in tiles])
    return idx_per_core, cols, row_order


def _prep_inputs(x, lengths, emb_table, W1, b1, W2, b2):
    x32 = np.asarray(x).astype(np.int32)
    idx_per_core, cols, row_order = _prep_idx(x32)

    lens = np.ascontiguousarray(
        np.asarray(lengths).astype(np.float32).reshape(B, 1)[row_order])
    # Fold W1 into the table: emb2 = emb @ W1 [V, H], padded fp16 to 256B rows
    emb2 = np.asarray(emb_table, dtype=np.float32) @ np.asarray(W1, dtype=np.float32)
    emb_p = np.zeros((V + 2, E), dtype=np.float16)
    emb_p[1:V + 1, :H] = emb2.astype(np.float16)
    b1c = np.ascontiguousarray(np.asarray(b1, dtype=np.float32).reshape(H, 1))
    w2 = np.ascontiguousarray(np.asarray(W2, dtype=np.float32))
    b2c = np.ascontiguousarray(np.asarray(b2, dtype=np.float32).reshape(C, 1))
    in_maps = [
        {
            **{f"idx{t}": idx_per_core[c][t] for t in range(N_TILES)},
            "lens": lens[c * BS:(c + 1) * BS],
            "emb": emb_p,
            "b1": b1c,
            "w2": w2,
            "b2": b2c,
        }
        for c in range(N_CORES)
    ]
    return in_maps, cols, row_order


def run_on_device(in_maps, cols, **kwargs):
    if _CACHE.get("key") != cols:
        _CACHE["nc"] = _build_nc(cols)
        _CACHE["key"] = cols
    return run_bass_kernel_spmd(_CACHE["nc"], in_maps, list(range(N_CORES)),
                                **kwargs)


def kernel(x, lengths, emb_table, W1, b1, W2, b2):
    in_maps, cols, row_order = _prep_inputs(
        x, lengths, emb_table, W1, b1, W2, b2)
    res = run_on_device(in_maps, cols)
    out = np.concatenate([r["out"] for r in res.results], axis=1)  # [C, B]
    full = np.empty((B, C), dtype=np.float32)
    full[row_order] = out.T  # undo the global row sort
    return full


# revision 18
# speedup vs baseline: 2.0170x; 1.5629x over previous
"""Trainium2 Bass kernel for BaselineDNN, W1-folded variant.

  logits[b] = relu((sum_l emb[x[b,l]]) / len[b] @ W1 + b1) @ W2 + b2

Key algebraic fold: pooling and the first MLP layer are both linear, so
  (sum_l emb[x[b,l]]) @ W1 = sum_l (emb @ W1)[x[b,l]].
The host precomputes emb2 = emb_table @ W1 [50000, 32] and the device
gathers 256B rows ([128] fp16, first 32 valid) instead of 768B rows of the
raw table — 3x less HBM gather traffic, same descriptor count.

Sharding: data-parallel over batch. Each of the 8 cores handles B/8 = 256
batch rows; the folded table and tiny tail weights are replicated. One SPMD
program runs on all 8 cores.

Gather: the dma_gather primitive takes int16 (signed) row indices, so the
50002-row table needs two base views:
  lo: rows [0, 32768)      idx = x + 1       (x <= 32766), filler idx 0
  hi: rows [17234, 50002)  idx = x - 17233   (x >= 32767), filler idx 32767
Row 0 and row 50001 are zero rows (fillers). Per batch row the 200 tokens
are partitioned host-side (order-invariant under the sum) into a lo-list
and a hi-list. Rows are globally sorted by lo-count into 16 narrow-spread
tiles of 128 (tile g pairs with tile 15-g on a core to balance work); the
host inverse-permutes the output.

HW datapath facts (microbenched): gather throughput is per-SWDGE-queue
bound at ~8.1 ns/descriptor and scales linearly up to the 4-queue ucode
max; address randomness and element size are second-order. So the gather
plan is: per core, split each tile's lo/hi block into 2 equal column
chunks -> 8 gathers, greedily assigned to the 4 queues so every queue
carries an equal descriptor load. Ring scratch is 64KB/partition so a
~80-column gather (5 descs/ring-slot..) fits its ring without stalling
descriptor generation.

Pooling: per valid token-slot, a TensorE matmul with a 128x128 fp16
identity accumulates the [128, 32] slot slice into fp32 PSUM. ScalarE
scales by 1/len, TensorE transposes to [32, 128], ScalarE applies b1/relu,
TensorE runs W2, ScalarE adds b2. Logits are written transposed [3, 256]
per core; the host reassembles and un-permutes [2048, 3].
"""

import numpy as np
from contextlib import ExitStack

import concourse.bass as bass
import concourse.bacc as bacc
import concourse.mybir as mybir
import concourse.tile as tile
from concourse.bass_utils import run_bass_kernel_spmd
from concourse.masks import make_identity

# Problem shapes (hardcoded per spec)
B, L, V, D, H, C = 2048, 200, 50000, 300, 32, 3
N_CORES = 8
BS = B // N_CORES   # 256 batch rows per core
P = 128             # partitions
N_TILES = BS // P   # batch tiles per core
E = 128             # folded fp16 row (256B, the dma_gather minimum)

CHUNK_TARGET = 20      # target gather width in 128-token columns (sweet spot:
                       # ~160 descs/ring keeps 3 gathers in a 512-desc ring)
SINGLE_PACKET = False  # True crashes the runtime on multi-packet gathers
DMA_SCRATCH = 32768    # SWDGE descriptor-ring carveout bytes/partition
N_QUEUES = 4           # SWDGE queues used (ucode max)
GBUFS = 8              # in-flight gather buffers
NO_COMPUTE = 0         # debug: skip matmuls/tail, gathers + idx DMAs only
TARGET_LO = 100        # per-row lo-token target (overlap rebalancing)

LO_BASE = 0         # lo view: table rows [0, 32768)
HI_BASE = 17234     # hi view: table rows [17234, 50002)
LO_FILL = 0         # zero row (table row 0)
HI_FILL = 32767     # zero row (table row 50001)
X_SPLIT = 32767     # x < split -> lo (idx x+1); x >= split -> hi (idx x-17233)
FILL_SENTINEL = -9999

F32 = mybir.dt.float32
F16 = mybir.dt.float16
I16 = mybir.dt.int16

_CACHE = {}


def _split_cols(total, target=None):
    """Split `total` columns into near-equal chunks of ~target width."""
    if target is None:
        target = CHUNK_TARGET
    n = max(1, (total + target - 1) // target)
    out = []
    for i in range(n):
        c = (total + (n - i) - 1) // (n - i)
        out.append(c)
        total -= c
    return [c for c in out if c > 0]


def _plan_jobs(cols):
    """cols[t] = (lo_cols, hi_cols) per tile. Returns a list of gather jobs
    (t, kind, col0, ncols, queue) in issue order, queues balanced by
    descriptor load."""
    jobs = []
    for t, (lo_c, hi_c) in enumerate(cols):
        for kind, tot in (("lo", lo_c), ("hi", hi_c)):
            c0 = 0
            for c in _split_cols(tot):
                jobs.append([t, kind, c0, c])
                c0 += c
    # greedy balance: biggest first onto the lightest queue
    order = sorted(range(len(jobs)), key=lambda j: -jobs[j][3])
    qload = [0] * N_QUEUES
    qassign = {}
    for j in order:
        q = min(range(N_QUEUES), key=lambda k: qload[k])
        qassign[j] = q
        qload[q] += jobs[j][3]
    # issue order: round-robin across queues so all start busy
    byq = [[j for j in range(len(jobs)) if qassign[j] == k] for k in range(N_QUEUES)]
    issue = []
    i = 0
    while any(byq):
        for k in range(N_QUEUES):
            if byq[k]:
                issue.append(byq[k].pop(0))
        i += 1
    return [(jobs[j][0], jobs[j][1], jobs[j][2], jobs[j][3], qassign[j])
            for j in issue], qload


def _build_nc(cols, reps=1):
    """cols[t] = (lo_cols, hi_cols): equalized valid 128-index columns per
    tile for the lo and hi blocks. Identical across cores by construction."""
    jobs, qload = _plan_jobs(cols)
    idx_cols = [8 * (lo + hi) for lo, hi in cols]

    nc = bacc.Bacc("TRN2", debug=False, num_devices=N_CORES,
                   num_swdge_queues=N_QUEUES,
                   dynamic_dma_scratch_size=DMA_SCRATCH)

    idx_ins = [
        nc.declare_dram_parameter(f"idx{t}", [P, idx_cols[t]], I16,
                                  isOutput=False)
        for t in range(N_TILES)
    ]
    len_in = nc.declare_dram_parameter("lens", [BS, 1], F32, isOutput=False)
    emb_in = nc.declare_dram_parameter("emb", [V + 2, E], F16, isOutput=False)
    b1_in = nc.declare_dram_parameter("b1", [H, 1], F32, isOutput=False)
    w2_in = nc.declare_dram_parameter("w2", [H, C], F32, isOutput=False)
    b2_in = nc.declare_dram_parameter("b2", [C, 1], F32, isOutput=False)
    out_dram = nc.declare_dram_parameter("out", [C, BS], F32, isOutput=True)

    emb_lo = emb_in[LO_BASE:LO_BASE + 32768, :]
    emb_hi = emb_in[HI_BASE:HI_BASE + 32768, :]

    with tile.TileContext(nc) as tc, ExitStack() as ctx:
        const_pool = ctx.enter_context(tc.tile_pool(name="const", bufs=1))
        xpool = ctx.enter_context(tc.tile_pool(name="xp", bufs=2))
        gpool = ctx.enter_context(tc.tile_pool(name="gp", bufs=GBUFS))
        spool = ctx.enter_context(tc.tile_pool(name="sp", bufs=2))
        psum_pool = ctx.enter_context(tc.tile_pool(name="ps", bufs=1, space="PSUM"))
        psum_acc = ctx.enter_context(tc.tile_pool(name="psacc", bufs=1, space="PSUM"))

        ident = const_pool.tile([P, P], F32)
        make_identity(nc, ident[:])
        ident16 = const_pool.tile([P, P], F16)
        make_identity(nc, ident16[:])
        b1_sb = const_pool.tile([H, 1], F32)
        nc.sync.dma_start(b1_sb[:], b1_in[:])
        w2_sb = const_pool.tile([H, C], F32)
        nc.sync.dma_start(w2_sb[:], w2_in[:])
        b2_sb = const_pool.tile([C, 1], F32)
        nc.sync.dma_start(b2_sb[:], b2_in[:])

        loop_ctx = tc.For_i(0, reps, 1) if reps > 1 else None
        if loop_ctx is not None:
            ctx.enter_context(loop_ctx)

        idx_t, lens_t, inv_t, acc, n_done = {}, {}, {}, {}, {}
        for t in range(N_TILES):
            r0 = t * P
            xt = xpool.tile([P, idx_cols[t]], I16, tag=f"xt{t}", name=f"xt{t}")
            nc.sync.dma_start(xt[:], idx_ins[t][:, :])
            idx_t[t] = xt
            lt = xpool.tile([P, 1], F32, tag=f"lt{t}", name=f"lt{t}")
            nc.sync.dma_start(lt[:], len_in[r0:r0 + P, :])
            it = xpool.tile([P, 1], F32, tag=f"it{t}", name=f"it{t}")
            nc.vector.reciprocal(it[:], lt[:])
            inv_t[t] = it
            acc[t] = psum_acc.tile([P, H], F32, tag=f"acc{t}", name=f"acc{t}")
            n_done[t] = 0

        maxc = max(j[3] for j in jobs)
        for (t, kind, c0, ncols, q) in jobs:
            lo_c, hi_c = cols[t]
            n_valid_tot = lo_c + hi_c
            src = emb_lo if kind == "lo" else emb_hi
            # column offset of this job inside the tile's idx tile
            base = 0 if kind == "lo" else lo_c
            col0 = 8 * (base + c0)
            g = gpool.tile([P, maxc * E], F16, tag="g", name="g")
            gv = g[:, :ncols * E].rearrange("p (c e) -> p c e", c=ncols, e=E)
            nc.gpsimd.dma_gather(
                out_ap=gv,
                in_ap=src,
                idxs_ap=idx_t[t][:, col0:col0 + 8 * ncols],
                num_idxs=P * ncols,
                num_idxs_reg=P * ncols,
                elem_size=E,
                single_packet=SINGLE_PACKET,
                queue_num=q,
            )
            if not NO_COMPUTE:
                for k in range(ncols):
                    nc.tensor.matmul(
                        out=acc[t][:],
                        lhsT=ident16[:],
                        rhs=gv[:, k, 0:H],
                        start=(n_done[t] == 0),
                        stop=(n_done[t] == n_valid_tot - 1),
                    )
                    n_done[t] += 1

        for t in range(N_TILES if not NO_COMPUTE else 0):
            r0 = t * P
            # rep = acc / len  (ScalarE: PSUM -> SBUF with per-partition scale)
            rep = spool.tile([P, H], F32, tag=f"rep{t}", name=f"rep{t}")
            nc.scalar.mul(rep[:], acc[t][:], inv_t[t][:, :1])

            # transpose to [H, P], then h = relu(rep + b1)
            tp = psum_pool.tile([P, P], F32, tag=f"tp{t}", name=f"tp{t}")
            nc.tensor.transpose(tp[:H, :], rep[:, 0:H], ident[:])
            h_sb = spool.tile([H, P], F32, tag=f"hsb{t}", name=f"hsb{t}")
            nc.scalar.activation(
                h_sb[:], tp[:H, :], mybir.ActivationFunctionType.Relu,
                bias=b1_sb[:, :1], scale=1.0,
            )

            # logits = h @ W2 + b2, as [C, P]
            o_psum = psum_pool.tile([C, P], F32, tag=f"o{t}", name=f"o{t}")
            nc.tensor.matmul(out=o_psum[:], lhsT=w2_sb[:], rhs=h_sb[:],
                             start=True, stop=True)
            logits_sb = spool.tile([C, P], F32, tag=f"lg{t}", name=f"lg{t}")
            nc.scalar.activation(
                logits_sb[:], o_psum[:], mybir.ActivationFunctionType.Identity,
                bias=b2_sb[:, :1], scale=1.0,
            )
            nc.sync.dma_start(out_dram[:, r0:r0 + P], logits_sb[:])

    nc.finalize()
    return nc


def _block_last(vals):
    """Last real flat position + 1 for `vals` ([P, K] with FILL_SENTINEL
    marking fillers), in j = col*128 + p order."""
    real = vals != FILL_SENTINEL
    if not real.any():
        return 0
    cc, pp = np.nonzero(real.T)
    return int((cc * P + pp).max() + 1)


def _wrap_block(blk, lead_fill, n_valid):
    """[P, C] block -> [P, 8*C] wrapped+replicated int16 idx tile.

    dma_gather maps flat index j -> partition j%128, column-group j//128,
    reading the flat list wrapped over 16 partitions (element j at partition
    j%16, column j//16), replicated across the eight 16-partition groups.

    Positions < n_valid that are fillers point at a zero row; positions
    >= n_valid are -1 (trimmed by the Q7: no descriptors, no traffic).
    """
    p, c = blk.shape
    flat = blk.T.reshape(-1).astype(np.int32).copy()
    flat[flat == FILL_SENTINEL] = lead_fill
    flat[n_valid:] = -1
    flat = flat.astype(np.int16)
    w = flat.reshape(8 * c, 16).T           # [16, 8*c]: element j at (j%16, j//16)
    return np.tile(w, (8, 1))               # replicate to 128 partitions


def _prep_idx(x32):
    """Split tokens lo/hi per row, globally sort rows by lo-count into
    narrow-spread tiles, equalize per-(tile-slot, block) valid columns
    across cores, and build wrapped idx tiles.

    Returns (idx arrays per core: list over tiles of [P, 8*(lo+hi)],
    cols, row_order) where row_order[c*BS + i] is the original batch row
    handled by core c, slot i.
    """
    # Token classes: lo-only x<17234, overlap 17234<=x<=32766 (reachable by
    # BOTH table views), hi-only x>32766. Assign overlap tokens to whichever
    # side brings each row to TARGET_LO lo-tokens: every row then splits
    # ~100/100, killing nearly all gather padding.
    cls = np.where(x32 < HI_BASE, 0, np.where(x32 <= X_SPLIT - 1, 1, 2))
    n_lo_only = (cls == 0).sum(axis=1)                # [B]
    n_mid = (cls == 1).sum(axis=1)
    k = np.clip(TARGET_LO - n_lo_only, 0, n_mid)
    n_lo = n_lo_only + k                              # actual lo count per row
    k_lo = int(n_lo.max())
    k_hi = int((L - n_lo).max())
    order = np.argsort(cls, axis=1, kind="stable")    # lo-only, overlap, hi-only
    xo = np.take_along_axis(x32, order, axis=1)       # [B, L] in class order

    colsr = np.arange(L)[None, :]
    lo_vals = np.where(colsr < n_lo[:, None], xo + 1, FILL_SENTINEL)
    hi_src = np.take_along_axis(
        xo, np.minimum(colsr + n_lo[:, None], L - 1), axis=1)
    hi_vals = np.where(colsr < (L - n_lo)[:, None], hi_src - 17233, FILL_SENTINEL)
    lo_all = lo_vals[:, :k_lo]
    hi_all = hi_vals[:, :k_hi]

    # Global sort by n_lo -> 16 tiles of 128 rows with narrow n_lo spread;
    # within a tile sort descending so block tails are maximally trimmable.
    # Pair tile g with tile 15-g on one core to balance per-core work.
    gorder = np.argsort(n_lo, kind="stable")
    n_gtiles = B // P
    gtiles = [gorder[i * P:(i + 1) * P] for i in range(n_gtiles)]
    gtiles = [t[np.argsort(-n_lo[t], kind="stable")] for t in gtiles]

    tile_rows = {}
    for c in range(N_CORES):
        for t, g in enumerate([c, n_gtiles - 1 - c]):
            tile_rows[(c, t)] = gtiles[g]

    # Equalized valid column counts per (tile-slot, lo/hi): max over cores,
    # rounded up to a full 128-index column so every gathered column is
    # completely written (pooling matmuls touch every valid column).
    cols = []
    for t in range(N_TILES):
        pair = []
        for vals_all in (lo_all, hi_all):
            m = 0
            for c in range(N_CORES):
                rows = tile_rows[(c, t)]
                m = max(m, _block_last(vals_all[rows]))
            pair.append((m + P - 1) // P)
        cols.append(tuple(pair))
    cols = tuple(cols)

    idx_per_core = []
    row_order = np.empty(B, dtype=np.int64)
    for c in range(N_CORES):
        tiles = []
        for t in range(N_TILES):
            rows = tile_rows[(c, t)]
            row_order[c * BS + t * P:c * BS + (t + 1) * P] = rows
            lo_c, hi_c = cols[t]
            blocks = [
                _wrap_block(lo_all[rows][:, :lo_c], LO_FILL, lo_c * P),
                _wrap_block(hi_all[rows][:, :hi_c], HI_FILL, hi_c * P),
            ]
            tiles.append(np.concatenate(blocks, axis=1))
        idx_per_core.append([np.ascontiguousarray(a) for a in tiles])
    return idx_per_core, cols, row_order


def _prep_inputs(x, lengths, emb_table, W1, b1, W2, b2):
    x32 = np.asarray(x).astype(np.int32)
    idx_per_core, cols, row_order = _prep_idx(x32)

    lens = np.ascontiguousarray(
        np.asarray(lengths).astype(np.float32).reshape(B, 1)[row_order])
    # Fold W1 into the table: emb2 = emb @ W1 [V, H], padded fp16 to 256B rows
    emb2 = np.asarray(emb_table, dtype=np.float32) @ np.asarray(W1, dtype=np.float32)
    emb_p = np.zeros((V + 2, E), dtype=np.float16)
    emb_p[1:V + 1, :H] = emb2.astype(np.float16)
    b1c = np.ascontiguousarray(np.asarray(b1, dtype=np.float32).reshape(H, 1))
    w2 = np.ascontiguousarray(np.asarray(W2, dtype=np.float32))
    b2c = np.ascontiguousarray(np.asarray(b2, dtype=np.float32).reshape(C, 1))
    in_maps = [
        {
            **{f"idx{t}": idx_per_core[c][t] for t in range(N_TILES)},
            "lens": lens[c * BS:(c + 1) * BS],
            "emb": emb_p,
            "b1": b1c,
            "w2": w2,
            "b2": b2c,
        }
        for c in range(N_CORES)
    ]
    return in_maps, cols, row_order


def run_on_device(in_maps, cols, **kwargs):
    if _CACHE.get("key") != cols:
        _CACHE["nc"] = _build_nc(cols)
        _CACHE["key"] = cols
    return run_bass_kernel_spmd(_CACHE["nc"], in_maps, list(range(N_CORES)),
                                **kwargs)


def kernel(x, lengths, emb_table, W1, b1, W2, b2):
    in_maps, cols, row_order = _prep_inputs(
        x, lengths, emb_table, W1, b1, W2, b2)
    res = run_on_device(in_maps, cols)
    out = np.concatenate([r["out"] for r in res.results], axis=1)  # [C, B]
    full = np.empty((B, C), dtype=np.float32)
    full[row_order] = out.T  # undo the global row sort
    return full
